# revision 7
# baseline (speedup 1.0000x reference)
"""Trainium2 Bass kernel for nn_LSTMDetachLayer (T=256, B=64, IN=H=1024).

Forward pass of an LSTM; stop_gradient/detach is a numerical no-op, so this
is a plain LSTM scan.

Sharding: gate/hidden-sharded across 8 cores. Core r owns hidden block r
(128 hidden units = 512 gate rows i/f/g/o). The sequential recurrence needs
the FULL h each step; each core broadcasts its h-block slice (transposed,
bf16, [128, 64]) to every core's SBUF gather buffer via remote_dma_broadcast
(SBUF->SBUF cross-core DMA with remote semaphore increments). Gather-slot k
on core r holds hidden block (r XOR k); the host packs w_hh / h0 per core in
matching XOR order so the SPMD program is fully uniform (no Switch).

Per step:  psum[64,512] = sum_k  hT_slot[k][128,64]^T @ whhT[k][128,512]
           gates = psum + xg_t ; cell elementwise ; h2 [64,128]
           PE-transpose h2 -> [128,64] -> bf16 -> broadcast into parity buf.
The x-part (x @ w_ih^T + b, all T) is a big parallel matmul done first into
DRAM, streamed back per step.
"""

import os
import sys
import numpy as np

sys.path.insert(0, "/opt/trn_rl_repo")

import ml_dtypes  # noqa: E402

T, B, IN, H = 256, 64, 1024, 1024
NC = 8           # cores
HL = H // NC     # 128 hidden per core
GL = 4 * HL      # 512 gate rows per core
KT = IN // 128   # 8 K tiles
BF16 = ml_dtypes.bfloat16

LAST_EXEC_NS = None


def _build(nc_mod, bass, tile, mybir):
    """Build the Bass graph (same program for all 8 cores)."""
    from contextlib import ExitStack

    nc = nc_mod
    f32 = mybir.dt.float32
    bf16 = mybir.dt.bfloat16

    # ---- I/O ----
    xT_d = nc.dram_tensor("xT", [IN, T * B], bf16, kind="ExternalInput")
    wihT_d = nc.dram_tensor("wihT", [128, KT, GL], bf16, kind="ExternalInput")
    bsum_d = nc.dram_tensor("bsum", [1, GL], bf16, kind="ExternalInput")
    whhT_d = nc.dram_tensor("whhT", [128, KT, GL], bf16, kind="ExternalInput")
    h0T_d = nc.dram_tensor("h0T", [128, NC, B], bf16, kind="ExternalInput")
    c0r_d = nc.dram_tensor("c0r", [B, HL], f32, kind="ExternalInput")
    id64_d = nc.dram_tensor("id64", [B, B], f32, kind="ExternalInput")
    out_d = nc.dram_tensor("outr", [T, B, HL], f32, kind="ExternalOutput")
    cT_d = nc.dram_tensor("cTr", [B, HL], f32, kind="ExternalOutput")
    debug_dump = os.environ.get("KERNEL_DUMP", "0") == "1"
    if debug_dump:
        dump_d = nc.dram_tensor("dump", [128, NC, B], mybir.dt.bfloat16,
                                kind="ExternalOutput")

    rsem = nc.alloc_semaphore("rsem")   # remote arrivals (+16/step: 8 senders x2)
    lsem = nc.alloc_semaphore("lsem")   # local send-complete (+16 per bcast => +128/step)
    csem = nc.alloc_semaphore("csem")   # bcast_src copy done (+1/step)
    psem = nc.alloc_semaphore("psem")   # descgen done (+8/step)

    with tile.TileContext(nc) as tc:
        with ExitStack() as ctx:
            const = ctx.enter_context(tc.tile_pool(name="const", bufs=1))
            state = ctx.enter_context(tc.tile_pool(name="state", bufs=1))
            xin = ctx.enter_context(tc.tile_pool(name="xin", bufs=3))
            xgo = ctx.enter_context(tc.tile_pool(name="xgo", bufs=3))
            xgp = ctx.enter_context(tc.tile_pool(name="xgp", bufs=4))
            work = ctx.enter_context(tc.tile_pool(name="work", bufs=3))
            ps_x = ctx.enter_context(tc.tile_pool(name="ps_x", bufs=2, space="PSUM"))
            ps_g = ctx.enter_context(tc.tile_pool(name="ps_g", bufs=2, space="PSUM"))
            ps_t = ctx.enter_context(tc.tile_pool(name="ps_t", bufs=2, space="PSUM"))
            dram = ctx.enter_context(tc.tile_pool(name="dram", bufs=1, space="DRAM"))

            # ---- constants / persistent state ----
            wih_sb = const.tile([128, KT, GL], bf16)
            nc.sync.dma_start(wih_sb[:], wihT_d[:])
            whh_sb = const.tile([128, KT, GL], bf16)
            nc.sync.dma_start(whh_sb[:], whhT_d[:])
            b_sb = const.tile([1, GL], bf16)
            nc.sync.dma_start(b_sb[:], bsum_d[:])
            id_sb = const.tile([B, B], f32)
            nc.sync.dma_start(id_sb[:], id64_d[:])
            ones_sb = const.tile([1, 128], bf16)
            nc.vector.memset(ones_sb[:], 1.0)

            hbuf = [state.tile([128, NC, B], bf16, name=f"hbuf{p}", tag=f"hbuf{p}") for p in (0, 1)]
            nc.sync.dma_start(hbuf[0][:], h0T_d[:])
            c_sb = state.tile([B, HL], f32, tag="c")
            nc.sync.dma_start(c_sb[:], c0r_d[:])
            bsrc = [state.tile([128, B], bf16, name=f"bsrc{p}", tag=f"bsrc{p}") for p in (0, 1)]

            xg_dram = dram.tile([T * B, GL], f32)

            # ================= Phase X: xg = x @ w_ih^T + b =================
            # out chunk m: rows m*128..m*128+127 of [T*B, GL]
            for m in range(T * B // 128):
                xt = xin.tile([128, KT, 128], bf16, tag="xt")
                # xT[k*128+p, m*128+c] -> xt[p, k, c]
                nc.sync.dma_start(
                    xt[:], xT_d.rearrange("(k p) n -> p k n", k=KT)[:, :, m * 128:(m + 1) * 128]
                )
                px = ps_x.tile([128, GL], f32, tag="px")
                for k in range(KT):
                    nc.tensor.matmul(px[:], xt[:, k, :], wih_sb[:, k, :],
                                     start=(k == 0), stop=False)
                nc.tensor.matmul(px[:], ones_sb[:], b_sb[:], start=False, stop=True)
                xg_sb = xgo.tile([128, GL], f32, tag="xg")
                nc.vector.tensor_copy(xg_sb[:], px[:])
                nc.sync.dma_start(xg_dram[m * 128:(m + 1) * 128, :], xg_sb[:])

            # ================= Phase R: recurrence =================
            rdests_all8 = [(0, k) for k in range(NC)]
            with tc.tile_critical(no_gpsimd_drain=True):
                pid = nc.gpsimd.partition_id()
            myslot = bass.ts(pid, 1)
            for t in range(T):
                p = t % 2
                pn = (t + 1) % 2

                xg_t = xgp.tile([B, GL], f32, tag="xgt")
                nc.sync.dma_start(xg_t[:], xg_dram[t * B:(t + 1) * B, :])

                # --- gates matmul (waits for remote slices inside critical) ---
                pg = ps_g.tile([B, GL], f32, tag="pg")
                with tc.tile_critical(no_gpsimd_drain=True):
                    if t > 0:
                        nc.tensor.wait_ge(rsem, 16 * t)
                    for k in range(KT):
                        nc.tensor.matmul(pg[:], hbuf[p][:, k, :], whh_sb[:, k, :],
                                         start=(k == 0), stop=(k == KT - 1))

                if debug_dump and t == 3:
                    nc.sync.dma_start(dump_d[:], hbuf[1][:])

                # --- cell ---
                g_sb = work.tile([B, GL], f32, tag="g")
                nc.vector.tensor_tensor(g_sb[:], pg[:], xg_t[:], op=mybir.AluOpType.add)
                a_sb = work.tile([B, GL], f32, tag="a")
                SIG = mybir.ActivationFunctionType.Sigmoid
                TANH = mybir.ActivationFunctionType.Tanh
                nc.scalar.activation(a_sb[:, 0 * HL:1 * HL], g_sb[:, 0 * HL:1 * HL], SIG)
                nc.scalar.activation(a_sb[:, 1 * HL:2 * HL], g_sb[:, 1 * HL:2 * HL], SIG)
                nc.scalar.activation(a_sb[:, 2 * HL:3 * HL], g_sb[:, 2 * HL:3 * HL], TANH)
                nc.scalar.activation(a_sb[:, 3 * HL:4 * HL], g_sb[:, 3 * HL:4 * HL], SIG)
                ig = work.tile([B, HL], f32, tag="ig")
                nc.vector.tensor_tensor(ig[:], a_sb[:, 0:HL], a_sb[:, 2 * HL:3 * HL],
                                        op=mybir.AluOpType.mult)
                fc = work.tile([B, HL], f32, tag="fc")
                nc.vector.tensor_tensor(fc[:], a_sb[:, HL:2 * HL], c_sb[:],
                                        op=mybir.AluOpType.mult)
                nc.vector.tensor_tensor(c_sb[:], ig[:], fc[:], op=mybir.AluOpType.add)
                tc_sb = work.tile([B, HL], f32, tag="tc")
                nc.scalar.activation(tc_sb[:], c_sb[:], TANH)
                h2 = work.tile([B, HL], f32, tag="h2")
                nc.vector.tensor_tensor(h2[:], a_sb[:, 3 * HL:4 * HL], tc_sb[:],
                                        op=mybir.AluOpType.mult)

                nc.sync.dma_start(out_d[t, :, :], h2[:])

                if t == T - 1:
                    break

                # --- transpose h2 -> [128, 64] ---
                ptr = ps_t.tile([HL, B], f32, tag="ptr")
                nc.tensor.transpose(ptr[:], h2[:], id_sb[:])

                # --- comm: copy->bf16, 8 uniform single-dest broadcasts ---
                with tc.tile_critical(no_gpsimd_drain=True):
                    if t >= 2:
                        nc.vector.wait_ge(lsem, 16 * (t - 1))
                    nc.vector.tensor_copy(bsrc[p][:], ptr[:]).then_inc(csem, 1)
                    nc.gpsimd.wait_ge(csem, t + 1)
                    nc.gpsimd.remote_dma_broadcast(
                        hbuf[pn][:, myslot, :], bsrc[p][:],
                        remote_sem=rsem, local_sem=lsem,
                        rdests=rdests_all8,
                    ).then_inc(psem, 1)
                    nc.gpsimd.wait_ge(psem, t + 1)
                    nc.gpsimd.trigger_dma(1)

            nc.sync.dma_start(cT_d[:], c_sb[:])

    nc.compile()
    return nc


def _prep_inputs(x, h0, c0, w_ih, w_hh, b_ih, b_hh):
    """Host-side shard/pack. Returns in_maps list of 8 dicts."""
    xT = np.ascontiguousarray(x.reshape(T * B, IN).T).astype(BF16)  # [IN, T*B]
    bsum = (b_ih + b_hh).astype(np.float32)
    id64 = np.eye(B, dtype=np.float32)
    in_maps = []
    for r in range(NC):
        rows = np.concatenate([np.arange(q * H + r * HL, q * H + (r + 1) * HL)
                               for q in range(4)])
        wih_r = w_ih[rows]            # [GL, IN]
        whh_r = w_hh[rows]            # [GL, H]
        # wihT packed [128, KT, GL]: [p,k,c] = w_ih_r[c, k*128+p]
        wihT = np.ascontiguousarray(
            wih_r.T.reshape(KT, 128, GL).transpose(1, 0, 2)).astype(BF16)
        # whhT packed with XOR slot order: [p,k,c] = w_hh_r[c, (r^k)*128+p]
        whhT = np.empty((128, KT, GL), dtype=BF16)
        h0T = np.empty((128, NC, B), dtype=BF16)
        for k in range(KT):
            blk = k
            whhT[:, k, :] = whh_r[:, blk * 128:(blk + 1) * 128].T.astype(BF16)
            h0T[:, k, :] = h0[:, blk * 128:(blk + 1) * 128].T.astype(BF16)
        in_maps.append({
            "xT": xT,
            "wihT": wihT,
            "bsum": bsum[rows].reshape(1, GL).astype(BF16),
            "whhT": whhT,
            "h0T": h0T,
            "c0r": np.ascontiguousarray(c0[:, r * HL:(r + 1) * HL]).astype(np.float32),
            "id64": id64,
        })
    return in_maps


def _bench_run(nc, in_maps):
    """Timed execution via the same PJRT path, inputs pre-staged on device."""
    global LAST_EXEC_NS
    import time
    import jax
    import numpy as np
    from jax.sharding import Mesh, PartitionSpec
    from jax.experimental.shard_map import shard_map
    from concourse import bass2jax, mybir

    bass2jax.install_neuronx_cc_hook()
    partition_name = nc.partition_id_tensor.name if nc.partition_id_tensor else None
    in_names, out_names, out_avals = [], [], []
    for alloc in nc.m.functions[0].allocations:
        if not isinstance(alloc, mybir.MemoryLocationSet):
            continue
        name = alloc.memorylocations[0].name
        if alloc.kind == "ExternalInput":
            if name != partition_name:
                in_names.append(name)
        elif alloc.kind == "ExternalOutput":
            out_names.append(name)
            out_avals.append(jax.core.ShapedArray(
                tuple(alloc.tensor_shape), mybir.dt.np(alloc.dtype)))
    n_params = len(in_names)
    all_in = in_names + out_names + ([partition_name] if partition_name else [])

    def _body(*args):
        operands = list(args)
        if partition_name is not None:
            operands.append(bass2jax.partition_id_tensor())
        return tuple(bass2jax._bass_exec_p.bind(
            *operands, out_avals=tuple(out_avals), in_names=tuple(all_in),
            out_names=tuple(out_names), lowering_input_output_aliases=(),
            sim_require_finite=True, sim_require_nnan=True, nc=nc))

    n = NC
    devices = jax.devices()[:n]
    mesh = Mesh(np.asarray(devices), ("core",))
    nout = len(out_names)
    fn = jax.jit(shard_map(_body, mesh=mesh,
                           in_specs=(PartitionSpec("core"),) * (n_params + nout),
                           out_specs=(PartitionSpec("core"),) * nout,
                           check_rep=False), keep_unused=True)
    concat_in = [np.concatenate([np.asarray(in_maps[c][k]) for c in range(n)], axis=0)
                 for k in in_names]
    concat_zeros = [np.zeros((n * a.shape[0], *a.shape[1:]), a.dtype)
                    for a in out_avals]
    sh = jax.sharding.NamedSharding(mesh, PartitionSpec("core"))
    dev_in = [jax.device_put(a, sh) for a in concat_in + concat_zeros]
    out = fn(*dev_in)
    jax.block_until_ready(out)
    times = []
    for _ in range(5):
        t0 = time.perf_counter()
        out = fn(*dev_in)
        jax.block_until_ready(out)
        times.append(time.perf_counter() - t0)
    LAST_EXEC_NS = int(min(times) * 1e9)
    print(f"bench times (ms): {[round(t*1e3, 3) for t in times]}")
    return [{name: np.asarray(out[i]).reshape(n, *out_avals[i].shape)[c]
             for i, name in enumerate(out_names)} for c in range(n)]


def kernel(x, h0, c0, w_ih, w_hh, b_ih, b_hh, detach_mask):
    global LAST_EXEC_NS
    import concourse.bass as bass
    import concourse.tile as tile
    from concourse import bacc, mybir
    from concourse.bass_utils import run_bass_kernel_spmd

    x = np.asarray(x, dtype=np.float32)
    h0 = np.asarray(h0, dtype=np.float32)
    c0 = np.asarray(c0, dtype=np.float32)
    w_ih = np.asarray(w_ih, dtype=np.float32)
    w_hh = np.asarray(w_hh, dtype=np.float32)
    b_ih = np.asarray(b_ih, dtype=np.float32)
    b_hh = np.asarray(b_hh, dtype=np.float32)

    nc = bacc.Bacc(None, num_devices=NC)
    _build(nc, bass, tile, mybir)
    in_maps = _prep_inputs(x, h0, c0, w_ih, w_hh, b_ih, b_hh)

    if os.environ.get("KERNEL_BENCH", "0") == "1":
        outs = _bench_run(nc, in_maps)
    else:
        res = run_bass_kernel_spmd(nc, in_maps, core_ids=list(range(NC)), trace=False)
        outs = res.results

    out = np.concatenate([outs[r]["outr"] for r in range(NC)], axis=2)
    cT = np.concatenate([outs[r]["cTr"] for r in range(NC)], axis=1)
    hT = out[-1].copy()
    return out.astype(np.float32), hT.astype(np.float32), cT.astype(np.float32)


# revision 8
# speedup vs baseline: 1.5080x; 1.5080x over previous
"""Trainium2 Bass kernel for nn_LSTMDetachLayer (T=256, B=64, IN=H=1024).

Forward pass of an LSTM; stop_gradient/detach is a numerical no-op, so this
is a plain LSTM scan.

Sharding: gate/hidden-sharded across 8 cores. Core r owns hidden block r
(128 hidden units = 512 gate rows i/f/g/o). The sequential recurrence needs
the FULL h each step; each core broadcasts its h-block slice (transposed,
bf16, [128, 64]) to every core's SBUF gather buffer via remote_dma_broadcast
(SBUF->SBUF cross-core DMA with remote semaphore increments). Gather-slot k
on core r holds hidden block (r XOR k); the host packs w_hh / h0 per core in
matching XOR order so the SPMD program is fully uniform (no Switch).

Per step:  psum[64,512] = sum_k  hT_slot[k][128,64]^T @ whhT[k][128,512]
           gates = psum + xg_t ; cell elementwise ; h2 [64,128]
           PE-transpose h2 -> [128,64] -> bf16 -> broadcast into parity buf.
The x-part (x @ w_ih^T + b, all T) is a big parallel matmul done first into
DRAM, streamed back per step.
"""

import os
import sys
import numpy as np

sys.path.insert(0, "/opt/trn_rl_repo")

import ml_dtypes  # noqa: E402

T, B, IN, H = 256, 64, 1024, 1024
NC = 8           # cores
HL = H // NC     # 128 hidden per core
GL = 4 * HL      # 512 gate rows per core
KT = IN // 128   # 8 K tiles
BF16 = ml_dtypes.bfloat16

LAST_EXEC_NS = None


def _build(nc_mod, bass, tile, mybir):
    """Build the Bass graph (same program for all 8 cores)."""
    from contextlib import ExitStack

    nc = nc_mod
    f32 = mybir.dt.float32
    bf16 = mybir.dt.bfloat16

    # ---- I/O ----
    xT_d = nc.dram_tensor("xT", [IN, T * B], bf16, kind="ExternalInput")
    wihT_d = nc.dram_tensor("wihT", [128, KT, GL], bf16, kind="ExternalInput")
    bsum_d = nc.dram_tensor("bsum", [1, GL], bf16, kind="ExternalInput")
    whhT_d = nc.dram_tensor("whhT", [128, KT, GL], bf16, kind="ExternalInput")
    h0T_d = nc.dram_tensor("h0T", [128, NC, B], bf16, kind="ExternalInput")
    c0r_d = nc.dram_tensor("c0r", [B, HL], f32, kind="ExternalInput")
    id64_d = nc.dram_tensor("id64", [B, B], f32, kind="ExternalInput")
    out_d = nc.dram_tensor("outr", [T, B, HL], f32, kind="ExternalOutput")
    cT_d = nc.dram_tensor("cTr", [B, HL], f32, kind="ExternalOutput")
    debug_dump = os.environ.get("KERNEL_DUMP", "0") == "1"
    if debug_dump:
        dump_d = nc.dram_tensor("dump", [128, NC, B], mybir.dt.bfloat16,
                                kind="ExternalOutput")

    rsem = nc.alloc_semaphore("rsem")   # remote arrivals (+16/step: 8 senders x2)
    lsem = nc.alloc_semaphore("lsem")   # local send-complete (+16 per bcast => +128/step)
    csem = nc.alloc_semaphore("csem")   # bcast_src copy done (+1/step)
    psem = nc.alloc_semaphore("psem")   # descgen done (+8/step)

    with tile.TileContext(nc) as tc:
        with ExitStack() as ctx:
            const = ctx.enter_context(tc.tile_pool(name="const", bufs=1))
            state = ctx.enter_context(tc.tile_pool(name="state", bufs=1))
            xin = ctx.enter_context(tc.tile_pool(name="xin", bufs=3))
            xgo = ctx.enter_context(tc.tile_pool(name="xgo", bufs=3))
            xgp = ctx.enter_context(tc.tile_pool(name="xgp", bufs=4))
            work = ctx.enter_context(tc.tile_pool(name="work", bufs=3))
            ps_x = ctx.enter_context(tc.tile_pool(name="ps_x", bufs=2, space="PSUM"))
            ps_g = ctx.enter_context(tc.tile_pool(name="ps_g", bufs=2, space="PSUM"))
            ps_t = ctx.enter_context(tc.tile_pool(name="ps_t", bufs=2, space="PSUM"))
            dram = ctx.enter_context(tc.tile_pool(name="dram", bufs=1, space="DRAM"))

            # ---- constants / persistent state ----
            wih_sb = const.tile([128, KT, GL], bf16)
            nc.sync.dma_start(wih_sb[:], wihT_d[:])
            whh_sb = const.tile([128, KT, GL], bf16)
            nc.sync.dma_start(whh_sb[:], whhT_d[:])
            b_sb = const.tile([1, GL], bf16)
            nc.sync.dma_start(b_sb[:], bsum_d[:])
            id_sb = const.tile([B, B], f32)
            nc.sync.dma_start(id_sb[:], id64_d[:])
            ones_sb = const.tile([1, 128], bf16)
            nc.vector.memset(ones_sb[:], 1.0)

            hbuf = [state.tile([128, NC, B], bf16, name=f"hbuf{p}", tag=f"hbuf{p}") for p in (0, 1)]
            nc.sync.dma_start(hbuf[0][:], h0T_d[:])
            c_sb = state.tile([B, HL], f32, tag="c")
            nc.sync.dma_start(c_sb[:], c0r_d[:])
            bsrc = [state.tile([128, B], bf16, name=f"bsrc{p}", tag=f"bsrc{p}") for p in (0, 1)]

            xg_dram = dram.tile([T * B, GL], f32)

            # ================= Phase X: xg = x @ w_ih^T + b =================
            # out chunk m: rows m*128..m*128+127 of [T*B, GL]
            for m in range(T * B // 128):
                xt = xin.tile([128, KT, 128], bf16, tag="xt")
                # xT[k*128+p, m*128+c] -> xt[p, k, c]
                nc.sync.dma_start(
                    xt[:], xT_d.rearrange("(k p) n -> p k n", k=KT)[:, :, m * 128:(m + 1) * 128]
                )
                px = ps_x.tile([128, GL], f32, tag="px")
                for k in range(KT):
                    nc.tensor.matmul(px[:], xt[:, k, :], wih_sb[:, k, :],
                                     start=(k == 0), stop=False)
                nc.tensor.matmul(px[:], ones_sb[:], b_sb[:], start=False, stop=True)
                xg_sb = xgo.tile([128, GL], f32, tag="xg")
                nc.vector.tensor_copy(xg_sb[:], px[:])
                nc.sync.dma_start(xg_dram[m * 128:(m + 1) * 128, :], xg_sb[:])

            # ================= Phase R: recurrence =================
            rdests_all8 = [(0, k) for k in range(NC)]
            with tc.tile_critical(no_gpsimd_drain=True):
                pid = nc.gpsimd.partition_id()
            myslot = bass.ts(pid, 1)
            for t in range(T):
                p = t % 2
                pn = (t + 1) % 2

                xg_t = xgp.tile([B, GL], f32, tag="xgt")
                nc.sync.dma_start(xg_t[:], xg_dram[t * B:(t + 1) * B, :])

                # --- gates matmul (arrival gated by wait inside prev comm crit) ---
                pg = ps_g.tile([B, GL], f32, tag="pg")
                for k in range(KT):
                    nc.tensor.matmul(pg[:], hbuf[p][:, k, :], whh_sb[:, k, :],
                                     start=(k == 0), stop=(k == KT - 1))

                if debug_dump and t == 3:
                    nc.sync.dma_start(dump_d[:], hbuf[1][:])

                # --- cell ---
                g_sb = work.tile([B, GL], f32, tag="g")
                nc.vector.tensor_tensor(g_sb[:], pg[:], xg_t[:], op=mybir.AluOpType.add)
                a_sb = work.tile([B, GL], f32, tag="a")
                SIG = mybir.ActivationFunctionType.Sigmoid
                TANH = mybir.ActivationFunctionType.Tanh
                nc.scalar.activation(a_sb[:, 0 * HL:1 * HL], g_sb[:, 0 * HL:1 * HL], SIG)
                nc.scalar.activation(a_sb[:, 1 * HL:2 * HL], g_sb[:, 1 * HL:2 * HL], SIG)
                nc.scalar.activation(a_sb[:, 2 * HL:3 * HL], g_sb[:, 2 * HL:3 * HL], TANH)
                nc.scalar.activation(a_sb[:, 3 * HL:4 * HL], g_sb[:, 3 * HL:4 * HL], SIG)
                ig = work.tile([B, HL], f32, tag="ig")
                nc.vector.tensor_tensor(ig[:], a_sb[:, 0:HL], a_sb[:, 2 * HL:3 * HL],
                                        op=mybir.AluOpType.mult)
                fc = work.tile([B, HL], f32, tag="fc")
                nc.vector.tensor_tensor(fc[:], a_sb[:, HL:2 * HL], c_sb[:],
                                        op=mybir.AluOpType.mult)
                nc.vector.tensor_tensor(c_sb[:], ig[:], fc[:], op=mybir.AluOpType.add)
                tc_sb = work.tile([B, HL], f32, tag="tc")
                nc.scalar.activation(tc_sb[:], c_sb[:], TANH)
                h2 = work.tile([B, HL], f32, tag="h2")
                nc.vector.tensor_tensor(h2[:], a_sb[:, 3 * HL:4 * HL], tc_sb[:],
                                        op=mybir.AluOpType.mult)

                nc.sync.dma_start(out_d[t, :, :], h2[:])

                if t == T - 1:
                    break

                # --- transpose h2 -> [128, 64] ---
                ptr = ps_t.tile([HL, B], f32, tag="ptr")
                nc.tensor.transpose(ptr[:], h2[:], id_sb[:])

                # --- comm: copy->bf16, 8 uniform single-dest broadcasts ---
                with tc.tile_critical(no_gpsimd_drain=True):
                    if t >= 2:
                        nc.vector.wait_ge(lsem, 16 * (t - 1))
                    nc.vector.tensor_copy(bsrc[p][:], ptr[:]).then_inc(csem, 1)
                    nc.gpsimd.wait_ge(csem, t + 1)
                    nc.gpsimd.remote_dma_broadcast(
                        hbuf[pn][:, myslot, :], bsrc[p][:],
                        remote_sem=rsem, local_sem=lsem,
                        rdests=rdests_all8,
                    ).then_inc(psem, 1)
                    nc.gpsimd.wait_ge(psem, t + 1)
                    nc.gpsimd.trigger_dma(1)
                    nc.tensor.wait_ge(rsem, 16 * (t + 1))

            nc.sync.dma_start(cT_d[:], c_sb[:])

    nc.compile()
    return nc


def _prep_inputs(x, h0, c0, w_ih, w_hh, b_ih, b_hh):
    """Host-side shard/pack. Returns in_maps list of 8 dicts."""
    xT = np.ascontiguousarray(x.reshape(T * B, IN).T).astype(BF16)  # [IN, T*B]
    bsum = (b_ih + b_hh).astype(np.float32)
    id64 = np.eye(B, dtype=np.float32)
    in_maps = []
    for r in range(NC):
        rows = np.concatenate([np.arange(q * H + r * HL, q * H + (r + 1) * HL)
                               for q in range(4)])
        wih_r = w_ih[rows]            # [GL, IN]
        whh_r = w_hh[rows]            # [GL, H]
        # wihT packed [128, KT, GL]: [p,k,c] = w_ih_r[c, k*128+p]
        wihT = np.ascontiguousarray(
            wih_r.T.reshape(KT, 128, GL).transpose(1, 0, 2)).astype(BF16)
        # whhT packed with XOR slot order: [p,k,c] = w_hh_r[c, (r^k)*128+p]
        whhT = np.empty((128, KT, GL), dtype=BF16)
        h0T = np.empty((128, NC, B), dtype=BF16)
        for k in range(KT):
            blk = k
            whhT[:, k, :] = whh_r[:, blk * 128:(blk + 1) * 128].T.astype(BF16)
            h0T[:, k, :] = h0[:, blk * 128:(blk + 1) * 128].T.astype(BF16)
        in_maps.append({
            "xT": xT,
            "wihT": wihT,
            "bsum": bsum[rows].reshape(1, GL).astype(BF16),
            "whhT": whhT,
            "h0T": h0T,
            "c0r": np.ascontiguousarray(c0[:, r * HL:(r + 1) * HL]).astype(np.float32),
            "id64": id64,
        })
    return in_maps


def _bench_run(nc, in_maps):
    """Timed execution via the same PJRT path, inputs pre-staged on device."""
    global LAST_EXEC_NS
    import time
    import jax
    import numpy as np
    from jax.sharding import Mesh, PartitionSpec
    from jax.experimental.shard_map import shard_map
    from concourse import bass2jax, mybir

    bass2jax.install_neuronx_cc_hook()
    partition_name = nc.partition_id_tensor.name if nc.partition_id_tensor else None
    in_names, out_names, out_avals = [], [], []
    for alloc in nc.m.functions[0].allocations:
        if not isinstance(alloc, mybir.MemoryLocationSet):
            continue
        name = alloc.memorylocations[0].name
        if alloc.kind == "ExternalInput":
            if name != partition_name:
                in_names.append(name)
        elif alloc.kind == "ExternalOutput":
            out_names.append(name)
            out_avals.append(jax.core.ShapedArray(
                tuple(alloc.tensor_shape), mybir.dt.np(alloc.dtype)))
    n_params = len(in_names)
    all_in = in_names + out_names + ([partition_name] if partition_name else [])

    def _body(*args):
        operands = list(args)
        if partition_name is not None:
            operands.append(bass2jax.partition_id_tensor())
        return tuple(bass2jax._bass_exec_p.bind(
            *operands, out_avals=tuple(out_avals), in_names=tuple(all_in),
            out_names=tuple(out_names), lowering_input_output_aliases=(),
            sim_require_finite=True, sim_require_nnan=True, nc=nc))

    n = NC
    devices = jax.devices()[:n]
    mesh = Mesh(np.asarray(devices), ("core",))
    nout = len(out_names)
    fn = jax.jit(shard_map(_body, mesh=mesh,
                           in_specs=(PartitionSpec("core"),) * (n_params + nout),
                           out_specs=(PartitionSpec("core"),) * nout,
                           check_rep=False), keep_unused=True)
    concat_in = [np.concatenate([np.asarray(in_maps[c][k]) for c in range(n)], axis=0)
                 for k in in_names]
    concat_zeros = [np.zeros((n * a.shape[0], *a.shape[1:]), a.dtype)
                    for a in out_avals]
    sh = jax.sharding.NamedSharding(mesh, PartitionSpec("core"))
    dev_in = [jax.device_put(a, sh) for a in concat_in + concat_zeros]
    out = fn(*dev_in)
    jax.block_until_ready(out)
    times = []
    for _ in range(5):
        t0 = time.perf_counter()
        out = fn(*dev_in)
        jax.block_until_ready(out)
        times.append(time.perf_counter() - t0)
    LAST_EXEC_NS = int(min(times) * 1e9)
    print(f"bench times (ms): {[round(t*1e3, 3) for t in times]}")
    return [{name: np.asarray(out[i]).reshape(n, *out_avals[i].shape)[c]
             for i, name in enumerate(out_names)} for c in range(n)]


def kernel(x, h0, c0, w_ih, w_hh, b_ih, b_hh, detach_mask):
    global LAST_EXEC_NS
    import concourse.bass as bass
    import concourse.tile as tile
    from concourse import bacc, mybir
    from concourse.bass_utils import run_bass_kernel_spmd

    x = np.asarray(x, dtype=np.float32)
    h0 = np.asarray(h0, dtype=np.float32)
    c0 = np.asarray(c0, dtype=np.float32)
    w_ih = np.asarray(w_ih, dtype=np.float32)
    w_hh = np.asarray(w_hh, dtype=np.float32)
    b_ih = np.asarray(b_ih, dtype=np.float32)
    b_hh = np.asarray(b_hh, dtype=np.float32)

    nc = bacc.Bacc(None, num_devices=NC)
    _build(nc, bass, tile, mybir)
    in_maps = _prep_inputs(x, h0, c0, w_ih, w_hh, b_ih, b_hh)

    if os.environ.get("KERNEL_BENCH", "0") == "1":
        outs = _bench_run(nc, in_maps)
    else:
        res = run_bass_kernel_spmd(nc, in_maps, core_ids=list(range(NC)), trace=False)
        outs = res.results

    out = np.concatenate([outs[r]["outr"] for r in range(NC)], axis=2)
    cT = np.concatenate([outs[r]["cTr"] for r in range(NC)], axis=1)
    hT = out[-1].copy()
    return out.astype(np.float32), hT.astype(np.float32), cT.astype(np.float32)
